# revision 8
# baseline (speedup 1.0000x reference)
"""Trainium2 Bass kernel for nn_KnnConstraint (ball-query KNN constraint loss).

Math (faithful to the reference):
  For each batch b and query point i: take the first K=20 points j (in index
  order) with ||x_i - x_j||^2 <= r^2, drop the first one (the "self" slot),
  keep up to 19 neighbors.  For each kept (i, j):
      cd = ||x_i - x_j||, nd = ||c_i - c_j||, w = exp(-0.1 * nd^2)
      term = sqrt((cd - nd)^2 * w + 1e-20) ~= |cd - nd| * exp(-0.05 * nd^2)
  loss = mean over all B*N*19 slots (invalid slots contribute sqrt(1e-20)).

Kernel strategy (8 NeuronCores, SPMD):
  core c handles batch b = c // 2, query-row half h = c % 2 (2048 rows).
  Per (128-row i-tile) x (512-col j-chunk):
    - TensorE: d2 tile via one augmented matmul  [-2x,-2y,-2z,1,sq]^T @ [x,y,z,sq,1]
      (same for canonical coords -> nd2)
    - DVE: within = (d2 <= r^2);  s = running in-ball count (tensor_tensor_scan);
      select mask m = within * (2 <= s <= 20)
    - ACT: cd = exp(0.5*ln(d2+1e-5)), nd = exp(0.5*ln(nd2+1e-5)), e = exp(-0.05*nd2)
      (single activation-table set: natural_log_exp_and_others)
    - DVE: term = |cd-nd|*e*m, accumulated per-row via accum_out
  Host combines the 8 partial sums + exact invalid-slot epsilon terms.
"""

import numpy as np

N = 4096
B = 4
HALF = 2048
K = 20
P = 128
CHUNK = 512
N_ITILES = HALF // P  # 16
N_CHUNKS = N // CHUNK  # 8
NCORES = 8
SLOTS = K - 1  # 19
EPS_D2 = 1.0e-5  # bias so ln() never sees <= 0 (PSUM cancellation noise ~3e-6)

_CACHE = {}


def _build_program(r2: float):
    import concourse.bass as bass  # noqa: F401
    import concourse.mybir as mybir
    from concourse import bacc
    from concourse.tile import TileContext

    f32 = mybir.dt.float32
    ALU = mybir.AluOpType
    ACT = mybir.ActivationFunctionType

    nc = bacc.Bacc(None, target_bir_lowering=False)
    # single input tensor (one DMA -> one semaphore wait on the first matmul):
    # cols [0:HALF] stat_cur | [HALF:HALF+N] mov_cur | [..:+HALF] stat_can | [..:+N] mov_can
    allin = nc.declare_dram_parameter("allin", [5, 2 * (HALF + N)], f32, isOutput=False)
    out = nc.declare_dram_parameter("out", [P, P + N_ITILES], f32, isOutput=True)

    with TileContext(nc) as tc:
        with (
            tc.tile_pool(name="const", bufs=1) as cpool,
            tc.tile_pool(name="work", bufs=3) as wpool,
            tc.tile_pool(name="spool", bufs=3) as spool,
            tc.tile_pool(name="psum", bufs=2, space="PSUM") as ppool,
        ):
            allin_sb = cpool.tile_from(allin[:, :])
            sc_sb = allin_sb[:, 0:HALF]
            mc_sb = allin_sb[:, HALF : HALF + N]
            sn_sb = allin_sb[:, HALF + N : 2 * HALF + N]
            mn_sb = allin_sb[:, 2 * HALF + N : 2 * (HALF + N)]

            acc = cpool.tile([P, P + N_ITILES], f32)
            nc.vector.memset(acc, 0.0)

            eps_bias = cpool.tile([P, 1], f32)
            nc.vector.memset(eps_bias, EPS_D2)
            import math
            ln_r2e = float(math.log(r2 + EPS_D2))

            for t in range(N_ITILES):
                s_prev = None
                for c in range(N_CHUNKS):
                    col = t * N_CHUNKS + c
                    psum_c = ppool.tile([P, CHUNK], f32, tag="psc")
                    psum_n = ppool.tile([P, CHUNK], f32, tag="psn")
                    nc.tensor.matmul(
                        psum_c,
                        sc_sb[:, t * P : (t + 1) * P],
                        mc_sb[:, c * CHUNK : (c + 1) * CHUNK],
                        start=True,
                        stop=True,
                    )
                    nc.tensor.matmul(
                        psum_n,
                        sn_sb[:, t * P : (t + 1) * P],
                        mn_sb[:, c * CHUNK : (c + 1) * CHUNK],
                        start=True,
                        stop=True,
                    )

                    # ACT reads PSUM exclusively (keeps every instruction at
                    # <=1 cross-engine wait: walrus allows one sync-wait slot).
                    lc = wpool.tile([P, CHUNK], f32, tag="lc")
                    nc.scalar.activation(lc, psum_c, ACT.Ln, bias=eps_bias[:, :], scale=1.0)
                    cd = wpool.tile([P, CHUNK], f32, tag="cd")
                    nc.scalar.activation(cd, lc, ACT.Exp, bias=0.0, scale=0.5)
                    l2 = wpool.tile([P, CHUNK], f32, tag="l2")
                    nc.scalar.activation(l2, psum_n, ACT.Ln, bias=eps_bias[:, :], scale=1.0)
                    nd = wpool.tile([P, CHUNK], f32, tag="nd")
                    nc.scalar.activation(nd, l2, ACT.Exp, bias=0.0, scale=0.5)
                    e = wpool.tile([P, CHUNK], f32, tag="e")
                    nc.scalar.activation(e, psum_n, ACT.Exp, bias=0.0, scale=-0.05)

                    # within = (d2 <= r2) via the monotone ln image: lc <= ln(r2+eps)
                    w01 = wpool.tile([P, CHUNK], f32, tag="w01")
                    nc.vector.tensor_scalar(w01, lc, ln_r2e, None, ALU.is_le)

                    s_t = spool.tile([P, CHUNK], f32, tag="scan")
                    init = 0.0 if s_prev is None else s_prev[:, CHUNK - 1 : CHUNK]
                    nc.vector.tensor_tensor_scan(
                        s_t, w01, w01, init, ALU.add, ALU.bypass
                    )
                    s_prev = s_t

                    # m = (1.5 <= s <= 20.5) * within  -> ranks 2..20
                    b1 = wpool.tile([P, CHUNK], f32, tag="b1")
                    nc.vector.tensor_scalar(b1, s_t, 1.5, None, ALU.is_ge)
                    mb = wpool.tile([P, CHUNK], f32, tag="mb")
                    nc.vector.scalar_tensor_tensor(mb, s_t, 20.5, b1, ALU.is_le, ALU.mult)
                    m = wpool.tile([P, CHUNK], f32, tag="m")
                    nc.gpsimd.tensor_tensor(m, mb, w01, ALU.mult)
                    em = wpool.tile([P, CHUNK], f32, tag="em")
                    nc.gpsimd.tensor_tensor(em, e, m, ALU.mult)

                    diff = wpool.tile([P, CHUNK], f32, tag="diff")
                    nc.vector.tensor_tensor(diff, cd, nd, ALU.subtract)
                    z = wpool.tile([P, CHUNK], f32, tag="z")
                    nc.vector.tensor_tensor(z, diff, em, ALU.mult)
                    # sum over the chunk of |diff|*e*m (e*m >= 0)
                    nc.vector.tensor_reduce(
                        acc[:, col : col + 1],
                        z,
                        axis=mybir.AxisListType.X,
                        op=ALU.add,
                        apply_absolute_value=True,
                    )
                # total in-ball count per row of this i-tile
                nc.vector.tensor_copy(
                    acc[:, P + t : P + t + 1], s_prev[:, CHUNK - 1 : CHUNK]
                )

            nc.default_dma_engine.dma_start(out[:, :], acc[:, :])
    nc.compile()
    return nc


def _prep_core_inputs(xyz, canno, core):
    b, h = core // 2, core % 2
    pts = xyz[b]  # [N, 3]
    sq = (pts * pts).sum(-1)
    ones = np.ones(N, np.float32)
    mov_cur = np.stack([pts[:, 0], pts[:, 1], pts[:, 2], sq, ones]).astype(np.float32)
    q = pts[h * HALF : (h + 1) * HALF]
    sqq = sq[h * HALF : (h + 1) * HALF]
    oq = np.ones(HALF, np.float32)
    stat_cur = np.stack(
        [-2.0 * q[:, 0], -2.0 * q[:, 1], -2.0 * q[:, 2], oq, sqq]
    ).astype(np.float32)

    csq = (canno * canno).sum(-1)
    mov_can = np.stack(
        [canno[:, 0], canno[:, 1], canno[:, 2], csq, ones]
    ).astype(np.float32)
    cq = canno[h * HALF : (h + 1) * HALF]
    csqq = csq[h * HALF : (h + 1) * HALF]
    stat_can = np.stack(
        [-2.0 * cq[:, 0], -2.0 * cq[:, 1], -2.0 * cq[:, 2], oq, csqq]
    ).astype(np.float32)
    allin = np.concatenate([stat_cur, mov_cur, stat_can, mov_can], axis=1)
    return {"allin": np.ascontiguousarray(allin.astype(np.float32))}


def _run(xyz, canno, r2, trace=False):
    from concourse.bass_utils import run_bass_kernel_spmd

    key = ("v1", float(r2))
    if key not in _CACHE:
        _CACHE[key] = _build_program(float(r2))
    nc = _CACHE[key]
    in_maps = [_prep_core_inputs(xyz, canno, c) for c in range(NCORES)]
    res = run_bass_kernel_spmd(nc, in_maps, list(range(NCORES)), trace=trace)
    return res


def kernel(xyz, canno_xyz, radius, _trace=False, _return_res=False):
    xyz = np.asarray(xyz, np.float32)
    canno = np.asarray(canno_xyz, np.float32)
    r2 = float(np.asarray(radius, np.float32)) ** 2

    res = _run(xyz, canno, r2, trace=_trace)

    total = 0.0
    n_valid = 0.0
    for c in range(NCORES):
        o = res.results[c]["out"].astype(np.float64)
        total += o[:, :P].sum()
        cnt = o[:, P : P + N_ITILES]  # total in-ball count per row
        n_valid += np.minimum(np.maximum(cnt - 1.0, 0.0), float(SLOTS)).sum()

    total_slots = B * N * SLOTS
    eps_term = float(np.sqrt(np.float64(np.float32(1e-20))))
    loss = (total + (total_slots - n_valid) * eps_term) / total_slots
    out = np.array(loss, dtype=np.float32)
    if _return_res:
        return out, res
    return out
